# revision 35
# baseline (speedup 1.0000x reference)
"""Trainium2 Bass kernel for nn_CnnBasedRnn (2-layer conv-RNN).

Math: each layer computes h_t = tanh(conv3x3_stride(2,1)(concat(x_t, h_{t-1})) + b).
Because the conv input is [x_t (rows 0..63); h_{t-1} (rows 64..127)] with row
stride 2, output row i taps concat rows 2i-1..2i+1:
  rows 0..31  <- x_t only                        (bulk "A-pass")
  row  i>=32  <- h_{t-1} rows 2i-65..2i-63       (cascade regions)
Region cascade: rows 32..47 need prev-step rows <=31 (bulk), rows 48..55 need
<=47, 56..59 need <=55, 60..61 need <=59, 62 needs <=61 -- all bulk passes.
Only row 63 self-recurses (taps prev row 63); solved by fixed-point sweeps
over the whole sequence: H <- tanh(dv + W[2] (x) shift_t(H)), contracting by
~sum|W[2,:]| per sweep (~0.15 here, so ~5-7 sweeps reach the bf16 noise floor).

Layout: SBUF partitions = (img(2) x j(64)); free dim = groups of 64 slots per
timestep: slot 0 = layer_input_t[row 63], slot 1+r = h_{t-1}[row r] for
r=0..62. Row 63 lives in separate contiguous H tiles (so its DMA is one
dense transfer, not 32k 4-byte descriptors, and so layer-2 work that doesn't
tap row 63 is not serialized behind the layer-1 fixed point).
Column (j) conv taps are banded 128x128 (block-diag over img) bf16 matmul
weights; row taps select slot columns via strided APs. All matmuls are bf16
(1 cycle/row vs fp32's 4): harness tolerance is 2e-2, bf16 noise ~1e-3.
Host pre-transposes x / post-transposes the output.
"""

import os
import numpy as np

B, L, D, NCORES = 16, 256, 64, 8
BS = B // NCORES          # images per core
TB = 64                   # timesteps per block
NBLK = L // TB
SLOT = 64                 # slots per timestep group in S tiles
WARM = bool(int(os.environ.get("BASS_WARM_FILLERS", "1")))


def _band(w3):
    """[64,64] banded matrix M[jin, jout] = w3[jin-jout+1] for |jin-jout|<=1."""
    M = np.zeros((D, D), np.float32)
    for dj in range(3):
        jout = np.arange(D)
        jin = jout + dj - 1
        m = (jin >= 0) & (jin < D)
        M[jin[m], jout[m]] = w3[dj]
    return M


def _bands_tensor(Wn):
    """[128, 6, 128]: for (l, di): block-diag over img of _band(Wn[l, di])."""
    out = np.zeros((128, 6, 128), np.float32)
    for l in range(2):
        for di in range(3):
            M = _band(Wn[l, di])
            out[0:64, l * 3 + di, 0:64] = M
            out[64:128, l * 3 + di, 64:128] = M
    return np.ascontiguousarray(out)


def _conv1d3(v, w3):
    out = (w3[1] * v).copy()
    out[..., :-1] += w3[2] * v[..., 1:]
    out[..., 1:] += w3[0] * v[..., :-1]
    return out


def _numpy_layer(xl, Wl, bl, n_iter):
    """Reference decomposition (for sweep-count estimation). xl: (b,L,D,D)."""
    nb = xl.shape[0]
    h = np.zeros((nb, L, D, D), np.float32)
    xpad = np.zeros((nb, L, D + 2, D), np.float32)
    xpad[:, :, 1:D + 1] = xl
    for i in range(32):
        acc = np.zeros((nb, L, D), np.float32)
        for di in range(3):
            acc = acc + _conv1d3(xpad[:, :, 2 * i + di], Wl[di])
        h[:, :, i] = np.tanh(acc + bl)

    def S_prev(slot):
        out = np.zeros((nb, L, D), np.float32)
        if slot == 0:
            out[:, :] = xl[:, :, 63]
        else:
            out[:, 1:] = h[:, :-1, slot - 1]
        return out

    for lo, hi in ((32, 47), (48, 55), (56, 59), (60, 61), (62, 62)):
        for i in range(lo, hi + 1):
            acc = np.zeros((nb, L, D), np.float32)
            for di in range(3):
                acc = acc + _conv1d3(S_prev(2 * i - 64 + di), Wl[di])
            h[:, :, i] = np.tanh(acc + bl)

    dv = bl + _conv1d3(S_prev(62), Wl[0]) + _conv1d3(S_prev(63), Wl[1])
    H = np.zeros((nb, L, D), np.float32)
    deltas = []
    for _ in range(n_iter):
        Hp = np.zeros((nb, L, D), np.float32)
        Hp[:, 1:] = H[:, :-1]
        Hn = np.tanh(dv + _conv1d3(Hp, Wl[2]))
        deltas.append(float(np.abs(Hn - H).max()))
        H = Hn
    h[:, :, 63] = H
    return h, deltas


def _estimate_sweeps(x, Wn, bn):
    """Run the decomposition on one image, count sweeps until the remaining
    fixed-point truncation is well below the bf16 noise floor (~1e-3)."""
    xs = x[:1].astype(np.float32)
    nits = []
    for l in range(2):
        xs_out, deltas = _numpy_layer(xs, Wn[l], bn[l], 24)
        nit = 24
        for k, d in enumerate(deltas):
            if d < 2e-3:
                nit = k
                break
        nits.append(min(24, max(3, nit + 1)))
        xs = xs_out
    return nits


def _build_bass(bn, nits):
    import concourse.bass as bass  # noqa: F401
    import concourse.bacc as bacc
    import concourse.mybir as mybir
    import concourse.tile as tile

    f32 = mybir.dt.float32
    bf16 = mybir.dt.bfloat16
    Tanh = mybir.ActivationFunctionType.Tanh

    nc = bacc.Bacc("TRN2", target_bir_lowering=False)
    xT = nc.dram_tensor("xT", [128, L, D], bf16, kind="ExternalInput")
    bands = nc.dram_tensor("bands", [128, 6, 128], bf16, kind="ExternalInput")
    # Raw dump of S[1] groups 1..L: slot 0 is junk (x row 63), slots 1..63 are
    # h2 rows 0..62. One fully-contiguous 32KB/partition store.
    outS = nc.dram_tensor("outS", [128, L, SLOT], bf16, kind="ExternalOutput")
    h2out = nc.dram_tensor("h2out", [128, L], bf16, kind="ExternalOutput")

    with tile.TileContext(nc) as tc:
        with (
            tc.tile_pool(name="persist", bufs=1) as persist,
            tc.tile_pool(name="hpool", bufs=1) as hpool,
            tc.tile_pool(name="ppool", bufs=3, space="PSUM") as ppool,
            tc.tile_pool(name="pbpool", bufs=1, space="PSUM") as pbpool,
        ):
            # weights ride the (otherwise idle) Activation DMA queue so they
            # land in parallel with the first x chunk
            bsb = persist.tile([128, 6, 128], bf16)
            nc.scalar.dma_start(out=bsb, in_=bands[:])

            # whole input sequence stays resident (32KB/partition). Loads
            # stay on ONE queue, in order, smallest-first: the DMA engines
            # share HBM bandwidth across queues, so the chunk gating the
            # first matmul must not compete with later chunks.
            xq = persist.tile([128, L, D], bf16, name="xq")
            for (c0, c1) in ((0, 16), (16, 32), (32, 64), (64, 128),
                             (128, 192), (192, 256)):
                nc.sync.dma_start(out=xq[:, c0:c1, :], in_=xT[:, c0:c1, :])

            def BD(l, di):
                return bsb[:, l * 3 + di, :]

            S = [persist.tile([128, L + 1, SLOT], bf16, name=f"S{i}")
                 for i in range(2)]
            # Row-63 sequences: H[l][g] = h_l[g-1, row 63]
            H = [hpool.tile([128, L + 1], bf16, name=f"Hrow{i}")
                 for i in range(2)]
            bias_t = [hpool.tile([128, 1], f32, name=f"bias{i}")
                      for i in range(2)]
            for i in range(2):
                nc.vector.memset(bias_t[i][:, :], float(bn[i]))
            # Fixed-point ping-pong buffers, one pair per layer
            Hp = [[hpool.tile([128, L + 1], bf16, name=f"Hp{l}{k}")
                   for k in range(2)] for l in range(2)]
            for l in range(2):
                nc.vector.memset(Hp[l][0][:, :], 0.0)
                nc.vector.memset(Hp[l][1][:, 0:1], 0.0)
                nc.vector.memset(S[l][:, 0, :], 0.0)
            # group L slot 0 of S[1] is stored (junk) but never written
            nc.vector.memset(S[1][:, L, 0:1], 0.0)

            def a1_pass(t0, nt):
                """Layer-1 rows 0..31 for timesteps [t0, t0+nt)."""
                # slot0[g=t] = x_t[row 63]
                nc.vector.tensor_copy(S[0][:, t0:t0 + nt, 0],
                                      xq[:, t0:t0 + nt, 63])
                pa = ppool.tile([128, nt, 32], f32, name="pa", tag="acc")
                for q in range(nt // 16):
                    lt = t0 + q * 16
                    r0 = xq[:, lt:lt + 16, 1:62:2]     # di=0, i=1..31
                    r1 = xq[:, lt:lt + 16, 0:63:2]     # di=1, i=0..31
                    r2 = xq[:, lt:lt + 16, 1:64:2]     # di=2, i=0..31
                    o = pa[:, q * 16:(q + 1) * 16, :]
                    nc.tensor.matmul(o, BD(0, 1), r1, start=True, stop=False)
                    nc.tensor.matmul(o, BD(0, 2), r2, start=False, stop=False)
                    nc.tensor.matmul(o[:, :, 1:32], BD(0, 0), r0,
                                     start=False, stop=True)
                nc.scalar.activation(
                    S[0][:, t0 + 1:t0 + nt + 1, 1:33], pa[:, :, :],
                    Tanh, bias=bias_t[0][:, :])

            def warm(n=2):
                """Junk matmuls into a rotating PSUM tile: keeps the PE's HAM
                activity monitor from re-throttling the clock to 1.2 GHz
                during activation-gated stalls. Results are never read."""
                pw = ppool.tile([128, 512], f32, name="pw", tag="acc")
                for k in range(n):
                    nc.tensor.matmul(pw, BD(0, 0), xq[:, k * 8:k * 8 + 8, :],
                                     start=True, stop=True)

            def a2a_pass(half):
                """Layer-2 rows 0..30 for a half-block (row 31 needs h1[63]
                and is handled by a2b_pass, so this does not wait on
                iterate(0))."""
                t0 = half * 32
                pa = ppool.tile([128, 32, 32], f32, name="pa2", tag="acc")
                for q in range(2):
                    gs = t0 + q * 16 + 1
                    r1 = S[0][:, gs:gs + 16, 1:62:2]   # di=1: rows 0..60 even
                    r2 = S[0][:, gs:gs + 16, 2:63:2]   # di=2: rows 1..61 odd
                    r0 = S[0][:, gs:gs + 16, 2:61:2]   # di=0: rows 1..59 odd
                    o = pa[:, q * 16:(q + 1) * 16, :]
                    nc.tensor.matmul(o[:, :, 0:31], BD(1, 1), r1,
                                     start=True, stop=False)
                    nc.tensor.matmul(o[:, :, 0:31], BD(1, 2), r2,
                                     start=False, stop=False)
                    nc.tensor.matmul(o[:, :, 1:31], BD(1, 0), r0,
                                     start=False, stop=True)
                nc.scalar.activation(
                    S[1][:, t0 + 1:t0 + 33, 1:32], pa[:, :, 0:31],
                    Tanh, bias=bias_t[1][:, :])

            def a2b_head():
                """Layer-2 row 31 taps h1 rows 61, 62, 63 = slots 62, 63, H1.
                The slot-62/63 contributions don't need the layer-1 fixed
                point, so they accumulate into a held PSUM bank before it;
                a2b_tail adds the H1 tap and activates after it."""
                pb = pbpool.tile([128, 256], f32, name="pb", tag="pb")
                nc.tensor.matmul(pb, BD(1, 0), S[0][:, 1:L + 1, 62],
                                 start=True, stop=False)
                nc.tensor.matmul(pb, BD(1, 1), S[0][:, 1:L + 1, 63],
                                 start=False, stop=False)
                return pb

            def a2b_tail(pb):
                nc.tensor.matmul(pb, BD(1, 2), H[0][:, 1:L + 1],
                                 start=False, stop=True)
                nc.scalar.activation(S[1][:, 1:L + 1, 32], pb,
                                     Tanh, bias=bias_t[1][:, :])

            def region_tile(l, ilo, ihi, NTmm, NTact, t0):
                """One PSUM tile of a cascade stage: rows ilo..ihi for
                timestep groups [t0, t0+NTact)."""
                Sl = S[l]
                n = ihi - ilo + 1
                pr = ppool.tile([128, NTact, n], f32, name="pr", tag="acc")
                for tm in range(0, NTact, NTmm):
                    for di in range(3):
                        s0 = 2 * ilo - 64 + di
                        rhs = Sl[:, t0 + tm:t0 + tm + NTmm,
                                 s0:s0 + 2 * n - 1:2]
                        nc.tensor.matmul(pr[:, tm:tm + NTmm, :],
                                         BD(l, di), rhs,
                                         start=(di == 0), stop=(di == 2))
                nc.scalar.activation(
                    Sl[:, t0 + 1:t0 + NTact + 1, 1 + ilo:2 + ihi],
                    pr[:, :, :], Tanh, bias=bias_t[l][:, :])

            def region_pass(l, split_last, skip_first=0):
                """Cascade rows 32..62, region-major over the full sequence:
                each region reads only previous regions' rows at t-1.
                NTmm: timesteps per matmul (NTmm*n = 512); NTact: per PSUM
                tile / activation. If split_last, the final (row 62) stage is
                done in 4 chunks so output stores can begin early."""
                Sl = S[l]
                stages = [(32, 47, 32, 64), (48, 55, 64, 128),
                          (56, 59, 128, 128), (60, 61, 128, 128)]
                if skip_first:
                    stages = stages[skip_first:]
                if split_last:
                    stages.append((62, 62, 64, 64))
                else:
                    stages.append((62, 62, 256, 256))
                for (ilo, ihi, NTmm, NTact) in stages:
                    n = ihi - ilo + 1
                    for t0 in range(0, L, NTact):
                        pr = ppool.tile([128, NTact, n], f32, name="pr",
                                        tag="acc")
                        for tm in range(0, NTact, NTmm):
                            for di in range(3):
                                s0 = 2 * ilo - 64 + di
                                rhs = Sl[:, t0 + tm:t0 + tm + NTmm,
                                         s0:s0 + 2 * n - 1:2]
                                nc.tensor.matmul(pr[:, tm:tm + NTmm, :],
                                                 BD(l, di), rhs,
                                                 start=(di == 0),
                                                 stop=(di == 2))
                        nc.scalar.activation(
                            Sl[:, t0 + 1:t0 + NTact + 1, 1 + ilo:2 + ihi],
                            pr[:, :, :], Tanh, bias=bias_t[l][:, :])
                        if split_last and ilo == 62:
                            # store this block's slice of the final output as
                            # soon as its last row is done (overlaps iterate)
                            nc.sync.dma_start(
                                out=outS[:, t0:t0 + NTact, :],
                                in_=S[1][:, t0 + 1:t0 + NTact + 1, :])
                    if WARM and ilo >= 56:
                        warm(2)

            def region_pass_wave(l):
                """Wavefront variant for the output layer: stage s of block b
                issues at diagonal d = s + b, so (a) every instruction's
                inputs were produced 1-2 diagonals earlier (the in-order PE
                queue never stalls long), and (b) block b's store issues
                right after its final stage -- the first store starts ~5us
                earlier than stage-major order allows, hiding most of the
                HBM store time under the remaining blocks' compute."""
                Sl = S[l]
                stages = [(32, 47, 32), (48, 55, 64), (56, 59, 64),
                          (60, 61, 64), (62, 62, 64)]
                NST = len(stages)
                for dgn in range(NST + NBLK - 1):
                    for s in range(min(dgn, NST - 1) + 1):
                        b = dgn - s
                        if not (0 <= b < NBLK):
                            continue
                        ilo, ihi, NTmm = stages[s]
                        n = ihi - ilo + 1
                        t0 = b * TB
                        pr = ppool.tile([128, TB, n], f32, name="pr",
                                        tag="acc")
                        for tm in range(0, TB, NTmm):
                            for di in range(3):
                                s0 = 2 * ilo - 64 + di
                                rhs = Sl[:, t0 + tm:t0 + tm + NTmm,
                                         s0:s0 + 2 * n - 1:2]
                                nc.tensor.matmul(pr[:, tm:tm + NTmm, :],
                                                 BD(l, di), rhs,
                                                 start=(di == 0),
                                                 stop=(di == 2))
                        nc.scalar.activation(
                            Sl[:, t0 + 1:t0 + TB + 1, 1 + ilo:2 + ihi],
                            pr[:, :, :], Tanh, bias=bias_t[l][:, :])
                        if s == NST - 1:
                            nc.sync.dma_start(
                                out=outS[:, t0:t0 + TB, :],
                                in_=S[1][:, t0 + 1:t0 + TB + 1, :])
                    if WARM and dgn >= NST:
                        warm(1)

            def iterate(l, fillers=(), per_sweep=2):
                """Fixed point for row 63; final sweep writes H[l].
                The two sweep-constant matmuls of sweep k are issued before
                the act-dependent third matmul of sweep k-1, and `fillers`
                (independent work, e.g. layer-2 A-pass tiles) are drizzled in
                so the PE stays busy (and the HAM clock stays warm) during
                each activation wait."""
                Sl = S[l]
                fillers = list(fillers)
                pend = None
                for k in range(nits[l]):
                    for _ in range(per_sweep):
                        if fillers:
                            fillers.pop(0)()
                    pi = ppool.tile([128, 256], f32, name=f"pi{k}", tag="acc")
                    nc.tensor.matmul(pi, BD(l, 0), Sl[:, 0:L, 62],
                                     start=True, stop=False)
                    nc.tensor.matmul(pi, BD(l, 1), Sl[:, 0:L, 63],
                                     start=False, stop=False)
                    if pend is not None:
                        pend()
                    def fin(k=k, pi=pi):
                        nc.tensor.matmul(pi, BD(l, 2), Hp[l][k % 2][:, 0:L],
                                         start=False, stop=True)
                        dst = (H[l][:, 1:L + 1] if k == nits[l] - 1
                               else Hp[l][(k + 1) % 2][:, 1:L + 1])
                        nc.scalar.activation(dst, pi, Tanh,
                                             bias=bias_t[l][:, :])
                    pend = fin
                pend()
                for f in fillers:
                    f()

            # ---- layer 1 ----
            # first two tiles are 16 timesteps so compute starts as soon as
            # the first small DMA chunk lands. Region stage 1 for block b
            # needs only a1 groups <= (b+1)*64, so it interleaves with the
            # (DMA-paced) a1 stream and fills the PE during chunk waits.
            a1_pass(0, 16)
            a1_pass(16, 16)
            for half in range(1, L // 32):
                a1_pass(half * 32, 32)
                if half % 2 == 1:
                    region_tile(0, 32, 47, 32, 64, (half // 2) * 64)
                if half == 5:
                    # stage 2 for groups 0..127 only needs stage-1 tiles
                    # t0=0,64 (issued at halves 1 and 3)
                    region_tile(0, 48, 55, 64, 128, 0)
            region_tile(0, 48, 55, 64, 128, 128)
            region_pass(0, split_last=False, skip_first=2)
            # ---- layer 2 ----
            # a2b's it0-independent taps accumulate up front; a2a only needs
            # region(0) output (h1 rows <= 62), not the row-63 fixed point,
            # so it interleaves with iterate(0)'s sweeps
            pb = a2b_head()
            iterate(0, fillers=[
                (lambda h=h: a2a_pass(h)) for h in range(L // 32)] +
                ([warm, warm] if WARM else []))
            a2b_tail(pb)
            # slot0[g] = h1_g[63]; group L's slot 0 is the memset above
            nc.vector.tensor_copy(S[1][:, 0:L, 0], H[0][:, 1:L + 1])
            if WARM:
                warm(3)
            region_pass_wave(1)
            iterate(1, fillers=[warm] * (3 if WARM else 0), per_sweep=1)
            # on the Activation queue: rides alongside the outS transfers
            # instead of queueing behind them on SP
            nc.scalar.dma_start(out=h2out[:, :], in_=H[1][:, 1:L + 1])

    nc.compile()
    return nc


def kernel(x, W, b):
    import sys
    if "/opt/trn_rl_repo" not in sys.path:
        sys.path.insert(0, "/opt/trn_rl_repo")
    from concourse.bass_utils import run_bass_kernel_spmd
    import ml_dtypes

    bfloat16 = ml_dtypes.bfloat16

    x = np.ascontiguousarray(np.asarray(x, np.float32))
    Wn = np.asarray(W, np.float32)[:, 0, 0]      # (2, 3, 3)
    bn = np.asarray(b, np.float32)               # (2,)

    nits = _estimate_sweeps(x, Wn, bn)
    nc = _build_bass(bn, nits)

    bands_np = _bands_tensor(Wn).astype(bfloat16)
    in_maps = []
    for c in range(NCORES):
        xc = x[c * BS:(c + 1) * BS]                      # (2, L, D, D)
        xTc = np.ascontiguousarray(
            xc.transpose(0, 3, 1, 2).reshape(128, L, D)).astype(bfloat16)
        in_maps.append({"xT": xTc, "bands": bands_np})

    res = run_bass_kernel_spmd(
        nc, in_maps, core_ids=list(range(NCORES)),
        trace=bool(int(os.environ.get("BASS_KERNEL_TRACE", "0"))))
    if os.environ.get("BASS_KERNEL_RESULT_PATH"):
        import pickle
        with open(os.environ["BASS_KERNEL_RESULT_PATH"], "wb") as f:
            pickle.dump({
                "exec_time_ns": res.exec_time_ns,
                "mean_exec_time_ns": res.mean_exec_time_ns,
                "trace": (res.instructions_and_trace or (None, None))[1],
                "profile_json": res.profile_json,
            }, f)

    out = np.empty((B, L, D, D), np.float32)
    for c in range(NCORES):
        r = res.results[c]
        main = np.asarray(r["outS"]).astype(np.float32)
        main = main.reshape(BS, D, L, SLOT)              # (img, j, t, slot)
        r63 = np.asarray(r["h2out"]).astype(np.float32).reshape(BS, D, L)
        out[c * BS:(c + 1) * BS, :, 0:63, :] = (
            main[:, :, :, 1:64].transpose(0, 2, 3, 1))
        out[c * BS:(c + 1) * BS, :, 63, :] = r63.transpose(0, 2, 1)
    return out


# revision 36
# speedup vs baseline: 1.0348x; 1.0348x over previous
"""Trainium2 Bass kernel for nn_CnnBasedRnn (2-layer conv-RNN).

Math: each layer computes h_t = tanh(conv3x3_stride(2,1)(concat(x_t, h_{t-1})) + b).
Because the conv input is [x_t (rows 0..63); h_{t-1} (rows 64..127)] with row
stride 2, output row i taps concat rows 2i-1..2i+1:
  rows 0..31  <- x_t only                        (bulk "A-pass")
  row  i>=32  <- h_{t-1} rows 2i-65..2i-63       (cascade regions)
Region cascade: rows 32..47 need prev-step rows <=31 (bulk), rows 48..55 need
<=47, 56..59 need <=55, 60..61 need <=59, 62 needs <=61 -- all bulk passes.
Only row 63 self-recurses (taps prev row 63); solved by fixed-point sweeps
over the whole sequence: H <- tanh(dv + W[2] (x) shift_t(H)), contracting by
~sum|W[2,:]| per sweep (~0.15 here, so ~5-7 sweeps reach the bf16 noise floor).

Layout: SBUF partitions = (img(2) x j(64)); free dim = groups of 64 slots per
timestep: slot 0 = layer_input_t[row 63], slot 1+r = h_{t-1}[row r] for
r=0..62. Row 63 lives in separate contiguous H tiles (so its DMA is one
dense transfer, not 32k 4-byte descriptors, and so layer-2 work that doesn't
tap row 63 is not serialized behind the layer-1 fixed point).
Column (j) conv taps are banded 128x128 (block-diag over img) bf16 matmul
weights; row taps select slot columns via strided APs. All matmuls are bf16
(1 cycle/row vs fp32's 4): harness tolerance is 2e-2, bf16 noise ~1e-3.
Host pre-transposes x / post-transposes the output.
"""

import os
import numpy as np

B, L, D, NCORES = 16, 256, 64, 8
BS = B // NCORES          # images per core
TB = 64                   # timesteps per block
NBLK = L // TB
SLOT = 64                 # slots per timestep group in S tiles
WARM = bool(int(os.environ.get("BASS_WARM_FILLERS", "1")))


def _band(w3):
    """[64,64] banded matrix M[jin, jout] = w3[jin-jout+1] for |jin-jout|<=1."""
    M = np.zeros((D, D), np.float32)
    for dj in range(3):
        jout = np.arange(D)
        jin = jout + dj - 1
        m = (jin >= 0) & (jin < D)
        M[jin[m], jout[m]] = w3[dj]
    return M


def _bands_tensor(Wn):
    """[128, 6, 128]: for (l, di): block-diag over img of _band(Wn[l, di])."""
    out = np.zeros((128, 6, 128), np.float32)
    for l in range(2):
        for di in range(3):
            M = _band(Wn[l, di])
            out[0:64, l * 3 + di, 0:64] = M
            out[64:128, l * 3 + di, 64:128] = M
    return np.ascontiguousarray(out)


def _conv1d3(v, w3):
    out = (w3[1] * v).copy()
    out[..., :-1] += w3[2] * v[..., 1:]
    out[..., 1:] += w3[0] * v[..., :-1]
    return out


def _numpy_layer(xl, Wl, bl, n_iter):
    """Reference decomposition (for sweep-count estimation). xl: (b,L,D,D)."""
    nb = xl.shape[0]
    h = np.zeros((nb, L, D, D), np.float32)
    xpad = np.zeros((nb, L, D + 2, D), np.float32)
    xpad[:, :, 1:D + 1] = xl
    for i in range(32):
        acc = np.zeros((nb, L, D), np.float32)
        for di in range(3):
            acc = acc + _conv1d3(xpad[:, :, 2 * i + di], Wl[di])
        h[:, :, i] = np.tanh(acc + bl)

    def S_prev(slot):
        out = np.zeros((nb, L, D), np.float32)
        if slot == 0:
            out[:, :] = xl[:, :, 63]
        else:
            out[:, 1:] = h[:, :-1, slot - 1]
        return out

    for lo, hi in ((32, 47), (48, 55), (56, 59), (60, 61), (62, 62)):
        for i in range(lo, hi + 1):
            acc = np.zeros((nb, L, D), np.float32)
            for di in range(3):
                acc = acc + _conv1d3(S_prev(2 * i - 64 + di), Wl[di])
            h[:, :, i] = np.tanh(acc + bl)

    dv = bl + _conv1d3(S_prev(62), Wl[0]) + _conv1d3(S_prev(63), Wl[1])
    H = np.zeros((nb, L, D), np.float32)
    deltas = []
    for _ in range(n_iter):
        Hp = np.zeros((nb, L, D), np.float32)
        Hp[:, 1:] = H[:, :-1]
        Hn = np.tanh(dv + _conv1d3(Hp, Wl[2]))
        deltas.append(float(np.abs(Hn - H).max()))
        H = Hn
    h[:, :, 63] = H
    return h, deltas


def _estimate_sweeps(x, Wn, bn):
    """Run the decomposition on one image, count sweeps until the remaining
    fixed-point truncation is well below the bf16 noise floor (~1e-3)."""
    xs = x[:1].astype(np.float32)
    nits = []
    for l in range(2):
        xs_out, deltas = _numpy_layer(xs, Wn[l], bn[l], 24)
        nit = 24
        for k, d in enumerate(deltas):
            if d < 2e-3:
                nit = k
                break
        nits.append(min(24, max(3, nit + 1)))
        xs = xs_out
    return nits


def _build_bass(bn, nits):
    import concourse.bass as bass  # noqa: F401
    import concourse.bacc as bacc
    import concourse.mybir as mybir
    import concourse.tile as tile

    f32 = mybir.dt.float32
    bf16 = mybir.dt.bfloat16
    Tanh = mybir.ActivationFunctionType.Tanh

    nc = bacc.Bacc("TRN2", target_bir_lowering=False)
    xT = nc.dram_tensor("xT", [128, L, D], bf16, kind="ExternalInput")
    bands = nc.dram_tensor("bands", [128, 6, 128], bf16, kind="ExternalInput")
    # Raw dump of S[1] groups 1..L: slot 0 is junk (x row 63), slots 1..63 are
    # h2 rows 0..62. One fully-contiguous 32KB/partition store.
    outS = nc.dram_tensor("outS", [128, L, SLOT], bf16, kind="ExternalOutput")
    h2out = nc.dram_tensor("h2out", [128, L], bf16, kind="ExternalOutput")

    with tile.TileContext(nc) as tc:
        with (
            tc.tile_pool(name="persist", bufs=1) as persist,
            tc.tile_pool(name="hpool", bufs=1) as hpool,
            tc.tile_pool(name="ppool", bufs=3, space="PSUM") as ppool,
            tc.tile_pool(name="pbpool", bufs=1, space="PSUM") as pbpool,
        ):
            # weights ride the (otherwise idle) Activation DMA queue so they
            # land in parallel with the first x chunk
            bsb = persist.tile([128, 6, 128], bf16)
            nc.scalar.dma_start(out=bsb, in_=bands[:])

            # whole input sequence stays resident (32KB/partition). Loads
            # stay on ONE queue, in order, smallest-first: the DMA engines
            # share HBM bandwidth across queues, so the chunk gating the
            # first matmul must not compete with later chunks.
            xq = persist.tile([128, L, D], bf16, name="xq")
            for (c0, c1) in ((0, 16), (16, 32), (32, 64), (64, 128),
                             (128, 192), (192, 256)):
                nc.sync.dma_start(out=xq[:, c0:c1, :], in_=xT[:, c0:c1, :])

            def BD(l, di):
                return bsb[:, l * 3 + di, :]

            S = [persist.tile([128, L + 1, SLOT], bf16, name=f"S{i}")
                 for i in range(2)]
            # Row-63 sequences: H[l][g] = h_l[g-1, row 63]
            H = [hpool.tile([128, L + 1], bf16, name=f"Hrow{i}")
                 for i in range(2)]
            bias_t = [hpool.tile([128, 1], f32, name=f"bias{i}")
                      for i in range(2)]
            for i in range(2):
                nc.vector.memset(bias_t[i][:, :], float(bn[i]))
            # Fixed-point ping-pong buffers, one pair per layer
            Hp = [[hpool.tile([128, L + 1], bf16, name=f"Hp{l}{k}")
                   for k in range(2)] for l in range(2)]
            for l in range(2):
                nc.vector.memset(Hp[l][0][:, :], 0.0)
                nc.vector.memset(Hp[l][1][:, 0:1], 0.0)
                nc.vector.memset(S[l][:, 0, :], 0.0)
            # group L slot 0 of S[1] is stored (junk) but never written
            nc.vector.memset(S[1][:, L, 0:1], 0.0)

            def a1_pass(t0, nt):
                """Layer-1 rows 0..31 for timesteps [t0, t0+nt)."""
                # slot0[g=t] = x_t[row 63]
                nc.vector.tensor_copy(S[0][:, t0:t0 + nt, 0],
                                      xq[:, t0:t0 + nt, 63])
                pa = ppool.tile([128, nt, 32], f32, name="pa", tag="acc")
                for q in range(nt // 16):
                    lt = t0 + q * 16
                    r0 = xq[:, lt:lt + 16, 1:62:2]     # di=0, i=1..31
                    r1 = xq[:, lt:lt + 16, 0:63:2]     # di=1, i=0..31
                    r2 = xq[:, lt:lt + 16, 1:64:2]     # di=2, i=0..31
                    o = pa[:, q * 16:(q + 1) * 16, :]
                    nc.tensor.matmul(o, BD(0, 1), r1, start=True, stop=False)
                    nc.tensor.matmul(o, BD(0, 2), r2, start=False, stop=False)
                    nc.tensor.matmul(o[:, :, 1:32], BD(0, 0), r0,
                                     start=False, stop=True)
                nc.scalar.activation(
                    S[0][:, t0 + 1:t0 + nt + 1, 1:33], pa[:, :, :],
                    Tanh, bias=bias_t[0][:, :])

            def warm(n=2):
                """Junk matmuls into a rotating PSUM tile: keeps the PE's HAM
                activity monitor from re-throttling the clock to 1.2 GHz
                during activation-gated stalls. Results are never read."""
                pw = ppool.tile([128, 512], f32, name="pw", tag="acc")
                for k in range(n):
                    nc.tensor.matmul(pw, BD(0, 0), xq[:, k * 8:k * 8 + 8, :],
                                     start=True, stop=True)

            def a2a_pass(half):
                """Layer-2 rows 0..30 for a half-block (row 31 needs h1[63]
                and is handled by a2b_pass, so this does not wait on
                iterate(0))."""
                t0 = half * 32
                pa = ppool.tile([128, 32, 32], f32, name="pa2", tag="acc")
                for q in range(2):
                    gs = t0 + q * 16 + 1
                    r1 = S[0][:, gs:gs + 16, 1:62:2]   # di=1: rows 0..60 even
                    r2 = S[0][:, gs:gs + 16, 2:63:2]   # di=2: rows 1..61 odd
                    r0 = S[0][:, gs:gs + 16, 2:61:2]   # di=0: rows 1..59 odd
                    o = pa[:, q * 16:(q + 1) * 16, :]
                    nc.tensor.matmul(o[:, :, 0:31], BD(1, 1), r1,
                                     start=True, stop=False)
                    nc.tensor.matmul(o[:, :, 0:31], BD(1, 2), r2,
                                     start=False, stop=False)
                    nc.tensor.matmul(o[:, :, 1:31], BD(1, 0), r0,
                                     start=False, stop=True)
                nc.scalar.activation(
                    S[1][:, t0 + 1:t0 + 33, 1:32], pa[:, :, 0:31],
                    Tanh, bias=bias_t[1][:, :])

            def a2b_head():
                """Layer-2 row 31 taps h1 rows 61, 62, 63 = slots 62, 63, H1.
                The slot-62/63 contributions don't need the layer-1 fixed
                point, so they accumulate into a held PSUM bank before it;
                a2b_tail adds the H1 tap and activates after it."""
                pb = pbpool.tile([128, 256], f32, name="pb", tag="pb")
                nc.tensor.matmul(pb, BD(1, 0), S[0][:, 1:L + 1, 62],
                                 start=True, stop=False)
                nc.tensor.matmul(pb, BD(1, 1), S[0][:, 1:L + 1, 63],
                                 start=False, stop=False)
                return pb

            def a2b_tail(pb):
                nc.tensor.matmul(pb, BD(1, 2), H[0][:, 1:L + 1],
                                 start=False, stop=True)
                nc.scalar.activation(S[1][:, 1:L + 1, 32], pb,
                                     Tanh, bias=bias_t[1][:, :])

            def region_tile(l, ilo, ihi, NTmm, NTact, t0):
                """One PSUM tile of a cascade stage: rows ilo..ihi for
                timestep groups [t0, t0+NTact)."""
                Sl = S[l]
                n = ihi - ilo + 1
                pr = ppool.tile([128, NTact, n], f32, name="pr", tag="acc")
                for tm in range(0, NTact, NTmm):
                    for di in range(3):
                        s0 = 2 * ilo - 64 + di
                        rhs = Sl[:, t0 + tm:t0 + tm + NTmm,
                                 s0:s0 + 2 * n - 1:2]
                        nc.tensor.matmul(pr[:, tm:tm + NTmm, :],
                                         BD(l, di), rhs,
                                         start=(di == 0), stop=(di == 2))
                nc.scalar.activation(
                    Sl[:, t0 + 1:t0 + NTact + 1, 1 + ilo:2 + ihi],
                    pr[:, :, :], Tanh, bias=bias_t[l][:, :])

            def region_pass(l, split_last, skip_first=False):
                """Cascade rows 32..62, region-major over the full sequence:
                each region reads only previous regions' rows at t-1.
                NTmm: timesteps per matmul (NTmm*n = 512); NTact: per PSUM
                tile / activation. If split_last, the final (row 62) stage is
                done in 4 chunks so output stores can begin early."""
                Sl = S[l]
                stages = [(32, 47, 32, 64), (48, 55, 64, 128),
                          (56, 59, 128, 128), (60, 61, 128, 128)]
                if skip_first:
                    stages = stages[1:]
                if split_last:
                    stages.append((62, 62, 64, 64))
                else:
                    stages.append((62, 62, 256, 256))
                for (ilo, ihi, NTmm, NTact) in stages:
                    n = ihi - ilo + 1
                    for t0 in range(0, L, NTact):
                        pr = ppool.tile([128, NTact, n], f32, name="pr",
                                        tag="acc")
                        for tm in range(0, NTact, NTmm):
                            for di in range(3):
                                s0 = 2 * ilo - 64 + di
                                rhs = Sl[:, t0 + tm:t0 + tm + NTmm,
                                         s0:s0 + 2 * n - 1:2]
                                nc.tensor.matmul(pr[:, tm:tm + NTmm, :],
                                                 BD(l, di), rhs,
                                                 start=(di == 0),
                                                 stop=(di == 2))
                        nc.scalar.activation(
                            Sl[:, t0 + 1:t0 + NTact + 1, 1 + ilo:2 + ihi],
                            pr[:, :, :], Tanh, bias=bias_t[l][:, :])
                        if split_last and ilo == 62:
                            # store this block's slice of the final output as
                            # soon as its last row is done (overlaps iterate)
                            nc.sync.dma_start(
                                out=outS[:, t0:t0 + NTact, :],
                                in_=S[1][:, t0 + 1:t0 + NTact + 1, :])
                    if WARM and ilo >= 56:
                        warm(2)

            def region_pass_wave(l):
                """Wavefront variant for the output layer: stage s of block b
                issues at diagonal d = s + b, so (a) every instruction's
                inputs were produced 1-2 diagonals earlier (the in-order PE
                queue never stalls long), and (b) block b's store issues
                right after its final stage -- the first store starts ~5us
                earlier than stage-major order allows, hiding most of the
                HBM store time under the remaining blocks' compute."""
                Sl = S[l]
                stages = [(32, 47, 32), (48, 55, 64), (56, 59, 64),
                          (60, 61, 64), (62, 62, 64)]
                NST = len(stages)
                for dgn in range(NST + NBLK - 1):
                    for s in range(min(dgn, NST - 1) + 1):
                        b = dgn - s
                        if not (0 <= b < NBLK):
                            continue
                        ilo, ihi, NTmm = stages[s]
                        n = ihi - ilo + 1
                        t0 = b * TB
                        pr = ppool.tile([128, TB, n], f32, name="pr",
                                        tag="acc")
                        for tm in range(0, TB, NTmm):
                            for di in range(3):
                                s0 = 2 * ilo - 64 + di
                                rhs = Sl[:, t0 + tm:t0 + tm + NTmm,
                                         s0:s0 + 2 * n - 1:2]
                                nc.tensor.matmul(pr[:, tm:tm + NTmm, :],
                                                 BD(l, di), rhs,
                                                 start=(di == 0),
                                                 stop=(di == 2))
                        nc.scalar.activation(
                            Sl[:, t0 + 1:t0 + TB + 1, 1 + ilo:2 + ihi],
                            pr[:, :, :], Tanh, bias=bias_t[l][:, :])
                        if s == NST - 1:
                            nc.sync.dma_start(
                                out=outS[:, t0:t0 + TB, :],
                                in_=S[1][:, t0 + 1:t0 + TB + 1, :])
                    if WARM and dgn >= NST:
                        warm(1)

            def iterate(l, fillers=(), per_sweep=2):
                """Fixed point for row 63; final sweep writes H[l].
                The two sweep-constant matmuls of sweep k are issued before
                the act-dependent third matmul of sweep k-1, and `fillers`
                (independent work, e.g. layer-2 A-pass tiles) are drizzled in
                so the PE stays busy (and the HAM clock stays warm) during
                each activation wait."""
                Sl = S[l]
                fillers = list(fillers)
                pend = None
                for k in range(nits[l]):
                    for _ in range(per_sweep):
                        if fillers:
                            fillers.pop(0)()
                    pi = ppool.tile([128, 256], f32, name=f"pi{k}", tag="acc")
                    nc.tensor.matmul(pi, BD(l, 0), Sl[:, 0:L, 62],
                                     start=True, stop=False)
                    nc.tensor.matmul(pi, BD(l, 1), Sl[:, 0:L, 63],
                                     start=False, stop=False)
                    if pend is not None:
                        pend()
                    def fin(k=k, pi=pi):
                        nc.tensor.matmul(pi, BD(l, 2), Hp[l][k % 2][:, 0:L],
                                         start=False, stop=True)
                        dst = (H[l][:, 1:L + 1] if k == nits[l] - 1
                               else Hp[l][(k + 1) % 2][:, 1:L + 1])
                        nc.scalar.activation(dst, pi, Tanh,
                                             bias=bias_t[l][:, :])
                    pend = fin
                pend()
                for f in fillers:
                    f()

            # ---- layer 1 ----
            # first two tiles are 16 timesteps so compute starts as soon as
            # the first small DMA chunk lands. Region stage 1 for block b
            # needs only a1 groups <= (b+1)*64, so it interleaves with the
            # (DMA-paced) a1 stream and fills the PE during chunk waits.
            a1_pass(0, 16)
            a1_pass(16, 16)
            for half in range(1, L // 32):
                a1_pass(half * 32, 32)
                if half % 2 == 1:
                    region_tile(0, 32, 47, 32, 64, (half // 2) * 64)
            region_pass(0, split_last=False, skip_first=True)
            # ---- layer 2 ----
            # a2b's it0-independent taps accumulate up front; a2a only needs
            # region(0) output (h1 rows <= 62), not the row-63 fixed point,
            # so it interleaves with iterate(0)'s sweeps
            pb = a2b_head()
            iterate(0, fillers=[
                (lambda h=h: a2a_pass(h)) for h in range(L // 32)] +
                ([warm, warm] if WARM else []))
            a2b_tail(pb)
            # slot0[g] = h1_g[63]; group L's slot 0 is the memset above
            nc.vector.tensor_copy(S[1][:, 0:L, 0], H[0][:, 1:L + 1])
            if WARM:
                warm(3)
            region_pass_wave(1)
            iterate(1, fillers=[warm] * (3 if WARM else 0), per_sweep=1)
            # on the Activation queue: rides alongside the outS transfers
            # instead of queueing behind them on SP
            nc.scalar.dma_start(out=h2out[:, :], in_=H[1][:, 1:L + 1])

    nc.compile()
    return nc


def kernel(x, W, b):
    import sys
    if "/opt/trn_rl_repo" not in sys.path:
        sys.path.insert(0, "/opt/trn_rl_repo")
    from concourse.bass_utils import run_bass_kernel_spmd
    import ml_dtypes

    bfloat16 = ml_dtypes.bfloat16

    x = np.ascontiguousarray(np.asarray(x, np.float32))
    Wn = np.asarray(W, np.float32)[:, 0, 0]      # (2, 3, 3)
    bn = np.asarray(b, np.float32)               # (2,)

    nits = _estimate_sweeps(x, Wn, bn)
    nc = _build_bass(bn, nits)

    bands_np = _bands_tensor(Wn).astype(bfloat16)
    in_maps = []
    for c in range(NCORES):
        xc = x[c * BS:(c + 1) * BS]                      # (2, L, D, D)
        xTc = np.ascontiguousarray(
            xc.transpose(0, 3, 1, 2).reshape(128, L, D)).astype(bfloat16)
        in_maps.append({"xT": xTc, "bands": bands_np})

    res = run_bass_kernel_spmd(
        nc, in_maps, core_ids=list(range(NCORES)),
        trace=bool(int(os.environ.get("BASS_KERNEL_TRACE", "0"))))
    if os.environ.get("BASS_KERNEL_RESULT_PATH"):
        import pickle
        with open(os.environ["BASS_KERNEL_RESULT_PATH"], "wb") as f:
            pickle.dump({
                "exec_time_ns": res.exec_time_ns,
                "mean_exec_time_ns": res.mean_exec_time_ns,
                "trace": (res.instructions_and_trace or (None, None))[1],
                "profile_json": res.profile_json,
            }, f)

    out = np.empty((B, L, D, D), np.float32)
    for c in range(NCORES):
        r = res.results[c]
        main = np.asarray(r["outS"]).astype(np.float32)
        main = main.reshape(BS, D, L, SLOT)              # (img, j, t, slot)
        r63 = np.asarray(r["h2out"]).astype(np.float32).reshape(BS, D, L)
        out[c * BS:(c + 1) * BS, :, 0:63, :] = (
            main[:, :, :, 1:64].transpose(0, 2, 3, 1))
        out[c * BS:(c + 1) * BS, :, 63, :] = r63.transpose(0, 2, 1)
    return out
